# revision 17
# baseline (speedup 1.0000x reference)
"""Trainium2 Bass kernel for nn_Net_66408784331557 (dense MLP with sync-BN).

Reference computation:
    h = BN_train(x; gamma_in, beta_in)            # x: [65536, 2048]
    h = relu(h @ W_in.T + b_in)                   # -> [65536, 75]
    12x: h = relu(BN_train(h; g_l, b_l) @ W_l.T + bias_l)
    out = h @ W_out.T + b_out                     # -> [65536, 1]

Strategy (v3): data-parallel over the batch across 8 NeuronCores (8192
rows each), shipping ONE fp16 layout xt [2048, 8192] per core (32 MB),
streamed twice.

Stream 1 (stats): per-feature sum/sumsq of x computed fully on-device
while DMA-bound: Scalar engine does Square+accum_out for most feature
blocks, DVE does tensor_tensor square (fp16 2x) + tensor_scalar
accumulate (fp16 4x mode) for the rest, plus all the x sums.  One
AllGather of [128, 16, 2] raw sums -> exact global mean/var -> fold
s = gamma*rsqrt(var+eps) into fp16 stationary weights + exact bias.

Stream 2 (matmul): re-stream xt, folded matmul (fp16 moving, 2
cols/cycle), ReLU+bias on Scalar with accum_out emitting sum(h1) for
free; DVE scalar_tensor_tensor gives sum(h1^2).  h kept in SBUF as
[75, 8192] f32r (fp16 storage would amplify through the 12-layer chain
~7x and blow the error budget).

Middle layers: per layer matmul (f32r) + ReLU/bias/accum (Scalar) +
sumsq (DVE stt) -> payload (sum, sumsq) [75+pad, 2].  Sync-BN exchange
either via ncfw AllGather (REMOTE=False) or direct SBUF->SBUF
remote_dma_broadcast peer writes with a post-schedule-patched
semaphore wait (REMOTE=True; Tile's scheduling sim cannot model
cross-core sem increments, so the wait is emitted as >=0 and patched
to the real target after scheduling).  Head fused into layer 12.
"""

import sys
import functools

import numpy as np

for _p in ("/opt/trn_rl_repo",):
    if _p not in sys.path:
        sys.path.insert(0, _p)

import ml_dtypes

N_CORES = 8
B = 65536
D = 2048
H = 75
L = 12
N_OUT = 1
EPS = 1e-5

F16 = np.float16

GW1 = 2048             # stream-2 group width (4 groups)
GWM = 1024             # middle-layer group width (8 groups)
XP_BUFS = 24           # stream-2 tile ring
REMOTE = False          # remote-dma stats exchange for middle layers
PREWARM_AR = True


def build_program(n_cores=N_CORES, b_local=B // N_CORES, d=D, h=H, n_layers=L,
                  remote=REMOTE, debug=False):
    """Builds the SPMD Bass/Tile program (identical on every core)."""
    import concourse.bass as bass
    import concourse.mybir as mybir
    import concourse.tile as tile
    import bass_rust as _bass_rust
    from concourse import bacc

    f32 = mybir.dt.float32
    f32r = mybir.dt.float32r
    f16 = mybir.dt.float16
    AF = mybir.ActivationFunctionType
    ALU = mybir.AluOpType
    AX = mybir.AxisListType

    QD = d // 128
    ng1 = b_local // GW1
    ngm = b_local // GWM
    B_TOT = n_cores * b_local
    remote = remote and n_cores == 8

    nc = bacc.Bacc("TRN2", target_bir_lowering=False, debug=debug,
                   enable_asserts=True, num_devices=n_cores)

    # ---- I/O ----
    xt_d = nc.dram_tensor("xt", [d, b_local], f16, kind="ExternalInput").ap()
    wint_d = nc.dram_tensor("wint", [128, QD, h], f32, kind="ExternalInput").ap()
    bin_d = nc.dram_tensor("bin", [h, 1], f32, kind="ExternalInput").ap()
    growp_d = nc.dram_tensor("growp", [128, QD], f32, kind="ExternalInput").ap()
    browp_d = nc.dram_tensor("browp", [128, QD], f32, kind="ExternalInput").ap()
    midwt_d = nc.dram_tensor("midwt", [n_layers, h, h], f32, kind="ExternalInput").ap()
    midg_d = nc.dram_tensor("midg", [h, n_layers], f32, kind="ExternalInput").ap()
    midbeta_d = nc.dram_tensor("midbeta", [h, n_layers], f32, kind="ExternalInput").ap()
    midbias_d = nc.dram_tensor("midbias", [h, n_layers], f32, kind="ExternalInput").ap()
    woutt_d = nc.dram_tensor("woutt", [h, N_OUT], f32, kind="ExternalInput").ap()
    bout_d = nc.dram_tensor("bout", [1, 1], f32, kind="ExternalInput").ap()
    identf_d = nc.dram_tensor("identf", [128, 128], f32, kind="ExternalInput").ap()
    sel16_d = nc.dram_tensor("sel16", [2 * n_cores, 2], f32, kind="ExternalInput").ap()
    out_d = nc.dram_tensor("out", [b_local, N_OUT], f32, kind="ExternalOutput").ap()

    rg = [list(range(n_cores))]

    rsem = nc.alloc_semaphore("rstats_sem") if remote else None
    lsem = nc.alloc_semaphore("rstats_lsem") if remote else None
    wait_patches = []  # (instruction, real wait target)

    with tile.TileContext(nc) as tc:
        with tc.tile_pool(name="const", bufs=1) as cp, \
             tc.tile_pool(name="drp", bufs=1, space="DRAM") as drp:

            # ---- constants into SBUF ----
            wint_sb = cp.tile([128, QD, h], f32)
            nc.sync.dma_start(wint_sb, wint_d)
            bin_sb = cp.tile([h, 1], f32)
            nc.sync.dma_start(bin_sb, bin_d)
            growp = cp.tile([128, QD], f32)
            nc.sync.dma_start(growp, growp_d)
            browp = cp.tile([128, QD], f32)
            nc.sync.dma_start(browp, browp_d)
            midwt_sb = cp.tile([h, n_layers, h], f32)
            nc.sync.dma_start(midwt_sb, midwt_d.rearrange("l k o -> k l o"))
            midg_sb = cp.tile([h, n_layers], f32)
            nc.sync.dma_start(midg_sb, midg_d)
            midbeta_sb = cp.tile([h, n_layers], f32)
            nc.sync.dma_start(midbeta_sb, midbeta_d)
            midbias_sb = cp.tile([h, n_layers], f32)
            nc.sync.dma_start(midbias_sb, midbias_d)
            woutt_sb = cp.tile([h, N_OUT], f32)
            nc.sync.dma_start(woutt_sb, woutt_d)
            bout_sb = cp.tile([1, 1], f32)
            nc.sync.dma_start(bout_sb, bout_d)
            identf = cp.tile([128, 128], f32)
            nc.sync.dma_start(identf, identf_d)
            sel16 = cp.tile([2 * n_cores, 2], f32)
            nc.sync.dma_start(sel16, sel16_d)

            if PREWARM_AR:
                wrm_i = drp.tile([1, 2], f32, name="wrm_i")
                wrm_o = drp.tile([n_cores, 1, 2], f32, name="wrm_o")
                nc.gpsimd.collective_compute(
                    "AllGather", mybir.AluOpType.bypass, replica_groups=rg,
                    ins=[wrm_i.opt()], outs=[wrm_o.opt()])

            # long-lived pools (released LIFO)
            hp = tc.alloc_tile_pool(name="hpool", bufs=1)
            h_a = hp.tile([h, b_local], f32r)
            h_b = hp.tile([h, b_local], f32r)
            sums = hp.tile([128, QD], f32)     # stream-1 per-q sum(x)
            sqs = hp.tile([128, QD], f32)      # stream-1 per-q sum(x^2)
            hs = hp.tile([h, ngm], f32)        # per-group sum(h)
            hq = hp.tile([h, ngm], f32)        # per-group sum(h^2)
            if remote:
                slots = hp.tile([128, n_layers, 8, 2], f32)
            sp = tc.alloc_tile_pool(name="small", bufs=2)
            scrp = tc.alloc_tile_pool(name="scrm", bufs=2)

            # =========== STREAM 1: per-feature sum/sumsq of x ============
            s1p = tc.alloc_tile_pool(name="s1p", bufs=3)
            s1s = tc.alloc_tile_pool(name="s1scr", bufs=2)
            s1s2 = tc.alloc_tile_pool(name="s1scr2", bufs=2)
            for q in range(QD):
                t = s1p.tile([128, b_local], f16, tag="x1", name=f"s1_{q}")
                nc.sync.dma_start(t, xt_d[q * 128:(q + 1) * 128, :])
                # sum(x) on DVE: tensor_scalar 4x mode, accum_out
                scb = s1s2.tile([128, b_local], f16, tag="sb", name=f"sb{q}")
                nc.vector.tensor_scalar(out=scb, in0=t, scalar1=1.0,
                                        scalar2=0.0, op0=ALU.mult,
                                        op1=ALU.add,
                                        accum_out=sums[:, q:q + 1])
                if q % 3 == 2:
                    # sumsq on DVE: square (2x) then accumulate (4x)
                    sca = s1s.tile([128, b_local], f16, tag="sa", name=f"sa{q}")
                    nc.vector.tensor_tensor(out=sca, in0=t, in1=t, op=ALU.mult)
                    nc.vector.tensor_scalar(out=sca, in0=sca, scalar1=1.0,
                                            scalar2=0.0, op0=ALU.mult,
                                            op1=ALU.add,
                                            accum_out=sqs[:, q:q + 1])
                else:
                    # sumsq on Scalar: Square activation with accum_out
                    sca = s1s.tile([128, b_local], f16, tag="sa", name=f"sa{q}")
                    nc.scalar.activation(sca, t, AF.Square,
                                         accum_out=sqs[:, q:q + 1])
            s1s2.release()
            s1s.release()
            s1p.release()

            # ---- AllGather raw sums; exact global stats; fold ----
            pay1 = sp.tile([128, QD, 2], f32, tag="pay1", bufs=1)
            nc.vector.tensor_copy(pay1[:, :, 0], sums)
            nc.vector.tensor_copy(pay1[:, :, 1], sqs)
            ag1i = drp.tile([128, QD * 2], f32, name="ag1i")
            ag1o = drp.tile([n_cores, 128, QD * 2], f32, name="ag1o")
            nc.scalar.dma_start(ag1i, pay1.rearrange("p q two -> p (q two)"))
            nc.gpsimd.collective_compute(
                "AllGather", mybir.AluOpType.bypass, replica_groups=rg,
                ins=[ag1i.opt()], outs=[ag1o.opt()])
            gx = sp.tile([128, n_cores, QD * 2], f32, tag="gx", bufs=1)
            nc.scalar.dma_start(gx, ag1o.rearrange("r p f -> p r f"))
            cur, k = gx, n_cores
            while k > 2:
                nxt = sp.tile([128, k // 2, QD * 2], f32, tag=f"ctr{k}", bufs=1)
                nc.vector.tensor_tensor(out=nxt, in0=cur[:, 0:k // 2, :],
                                        in1=cur[:, k // 2:k, :], op=ALU.add)
                cur, k = nxt, k // 2
            c1 = sp.tile([128, QD, 2], f32, tag="c1", bufs=1)
            nc.vector.tensor_tensor(
                out=c1.rearrange("p q two -> p (q two)"),
                in0=cur[:, 0, :], in1=cur[:, 1, :], op=ALU.add)
            # mu = S1/B ; var = S2/B - mu^2 ; s = gamma*rsqrt(var+eps)
            mu = sp.tile([128, QD], f32, tag="mu", bufs=1)
            nc.vector.tensor_scalar_mul(mu, c1[:, :, 0], 1.0 / B_TOT)
            vep = sp.tile([128, QD], f32, tag="vep", bufs=1)
            nc.vector.tensor_scalar(out=vep, in0=c1[:, :, 1],
                                    scalar1=1.0 / B_TOT, scalar2=float(EPS),
                                    op0=ALU.mult, op1=ALU.add)
            musq = sp.tile([128, QD], f32, tag="musq", bufs=1)
            nc.vector.tensor_tensor(out=musq, in0=mu, in1=mu, op=ALU.mult)
            nc.vector.tensor_tensor(out=vep, in0=vep, in1=musq, op=ALU.subtract)
            sd1 = sp.tile([128, QD], f32, tag="sd1", bufs=1)
            nc.scalar.activation(sd1, vep, AF.Sqrt)
            rr1 = sp.tile([128, QD], f32, tag="rr1", bufs=1)
            nc.vector.reciprocal(rr1, sd1)
            s_p = sp.tile([128, QD], f32, tag="s_p", bufs=1)
            nc.vector.tensor_tensor(out=s_p, in0=rr1, in1=growp, op=ALU.mult)
            t_p = sp.tile([128, QD], f32, tag="t_p", bufs=1)
            nc.vector.tensor_tensor(out=t_p, in0=mu, in1=s_p, op=ALU.mult)
            nc.vector.tensor_tensor(out=t_p, in0=browp, in1=t_p, op=ALU.subtract)
            wfold = cp.tile([128, QD, h], f16)
            for q in range(QD):
                nc.vector.tensor_scalar_mul(wfold[:, q, :], wint_sb[:, q, :],
                                            s_p[:, q:q + 1])
            bias1 = sp.tile([h, 1], f32, tag="bias1", bufs=1)
            with tc.tile_pool(name="pbias", bufs=1, space="PSUM") as pbias:
                pdum = pbias.tile([1, 1], f32, tag="pdum")
                nc.tensor.matmul(pdum, identf[0:128, 0:1], identf[0:128, 0:1],
                                 skip_group_check=True)
                ps_b1 = pbias.tile([h, 1], f32, tag="psb1")
                for q in range(QD):
                    nc.tensor.matmul(ps_b1, wint_sb[:, q, :], t_p[:, q:q + 1],
                                     start=(q == 0), stop=(q == QD - 1),
                                     skip_group_check=True)
                nc.vector.tensor_tensor(out=bias1, in0=ps_b1, in1=bin_sb,
                                        op=ALU.add)

            # =========== STREAM 2: h1 = relu(xn @ wfold + bias1) =========
            h_b_f = h_b.bitcast(f32)
            xp = tc.alloc_tile_pool(name="xp", bufs=XP_BUFS)
            with tc.tile_pool(name="p2ps", bufs=2, space="PSUM") as p2ps:
                for g in range(ng1):
                    tiles = []
                    for q in range(QD):
                        t = xp.tile([128, GW1], f16, tag="x2", name=f"s2_{g}_{q}")
                        nc.sync.dma_start(t, xt_d[q * 128:(q + 1) * 128,
                                                  g * GW1:(g + 1) * GW1])
                        tiles.append(t)
                    psy = p2ps.tile([h, GW1], f32, tag="psy", name=f"psy{g}")
                    for q in range(QD):
                        for c in range(GW1 // 512):
                            nc.tensor.matmul(psy[:, c * 512:(c + 1) * 512],
                                             wfold[:, q, :],
                                             tiles[q][:, c * 512:(c + 1) * 512],
                                             start=(q == 0), stop=(q == QD - 1),
                                             skip_group_check=True)
                    for gg in range(GW1 // GWM):
                        g2 = g * (GW1 // GWM) + gg
                        sl = slice(g2 * GWM, (g2 + 1) * GWM)
                        psl = slice(gg * GWM, (gg + 1) * GWM)
                        nc.scalar.activation(h_b[:, sl], psy[:, psl], AF.Relu,
                                             bias=bias1,
                                             accum_out=hs[:, g2:g2 + 1])
                        scr = scrp.tile([h, GWM], f32, tag="sq",
                                        name=f"sqh1_{g2}")
                        nc.vector.scalar_tensor_tensor(
                            out=scr, in0=h_b_f[:, sl], scalar=1.0,
                            in1=h_b_f[:, sl], op0=ALU.mult, op1=ALU.mult,
                            accum_out=hq[:, g2:g2 + 1])
            xp.release()

            # =========== 12 middle layers (+ head fused into last) =======
            h_in, h_out = h_b, h_a
            with tc.tile_pool(name="mid", bufs=2) as mp_, \
                 tc.tile_pool(name="midps", bufs=2, space="PSUM") as mps, \
                 tc.tile_pool(name="midpso", bufs=2, space="PSUM") as mpso, \
                 tc.tile_pool(name="midpb", bufs=1, space="PSUM") as mpb:
                out_row = mp_.tile([1, b_local], f32, bufs=1)
                woutt16 = mp_.tile([h, N_OUT], f32r, bufs=1)
                nc.vector.tensor_copy(woutt16, woutt_sb)

                for l in range(n_layers):
                    # ---- payload: (sum, sumsq) of h_in across local batch
                    if remote:
                        pay2 = mp_.tile([128, 2], f32, tag="pay", bufs=n_layers,
                                        name=f"pay{l}")
                    else:
                        pay2 = mp_.tile([h, 2], f32, tag="pay", name=f"pay{l}")
                    nc.vector.tensor_reduce(out=pay2[0:h, 0:1], in_=hs,
                                            axis=AX.X, op=ALU.add)
                    nc.vector.tensor_reduce(out=pay2[0:h, 1:2], in_=hq,
                                            axis=AX.X, op=ALU.add)

                    dg = mp_.tile([h, 2], f32, tag="dg", name=f"dg{l}")
                    if remote:
                        for dlt in range(1, 8):
                            rdests = [None] * 8
                            rdests[dlt] = (0, dlt)
                            nc.gpsimd.remote_dma_broadcast(
                                slots[:, l, dlt, :], pay2, rsem, lsem,
                                rdests=rdests)
                        nc.gpsimd.trigger_dma(count=None)
                        nc.vector.tensor_copy(slots[0:h, l, 0, :], pay2[0:h, :])
                        w_ins = nc.vector.wait_ge(rsem, 0).ins
                        wait_patches.append((w_ins, 14 * (l + 1)))
                        cm4 = mp_.tile([h, 4, 2], f32, tag="cm4", name=f"cm4{l}")
                        a1 = nc.vector.tensor_tensor(out=cm4,
                                                     in0=slots[0:h, l, 0:4, :],
                                                     in1=slots[0:h, l, 4:8, :],
                                                     op=ALU.add).ins
                        a1.add_dependency(w_ins.name,
                                          _bass_rust.DependencyInfo.NO_SYNC_ONLY)
                        cm2 = mp_.tile([h, 2, 2], f32, tag="cm2", name=f"cm2{l}")
                        nc.vector.tensor_tensor(out=cm2, in0=cm4[:, 0:2, :],
                                                in1=cm4[:, 2:4, :], op=ALU.add)
                        nc.vector.tensor_scalar_mul(dg, cm2[:, 0, :], 1.0 / B_TOT)
                        nc.vector.scalar_tensor_tensor(
                            out=dg, in0=cm2[:, 1, :], scalar=1.0 / B_TOT,
                            in1=dg, op0=ALU.mult, op1=ALU.add)
                    else:
                        ptp = mpb.tile([2, h], f32, tag="scr", name=f"ptp{l}")
                        nc.tensor.matmul(ptp, pay2, identf[0:h, 0:h],
                                         is_transpose=True, skip_group_check=True)
                        payT = mp_.tile([2, h], f32, tag="payT", name=f"payT{l}")
                        nc.vector.tensor_copy(payT, ptp)
                        agi = drp.tile([2, h], f32, name=f"agi{l}")
                        ago = drp.tile([n_cores, 2, h], f32, name=f"ago{l}")
                        nc.scalar.dma_start(agi, payT)
                        nc.gpsimd.collective_compute(
                            "AllGather", mybir.AluOpType.bypass,
                            replica_groups=rg,
                            ins=[agi.opt()], outs=[ago.opt()])
                        gT = mp_.tile([2 * n_cores, h], f32, tag="gT",
                                      name=f"gT{l}")
                        nc.scalar.dma_start(gT, ago.rearrange("r two h -> (r two) h"))
                        pdg = mpb.tile([h, 2], f32, tag="scr", name=f"pdg{l}")
                        nc.tensor.matmul(pdg, gT, sel16, skip_group_check=True)
                        nc.vector.tensor_scalar_mul(dg, pdg, 1.0 / B_TOT)

                    # ---- fold: s2, wf, bias2 (dg = (mean*?, Exsq*?)) ----
                    musq2 = mp_.tile([h, 1], f32, tag="musq2", name=f"musq2{l}")
                    nc.vector.tensor_tensor(out=musq2, in0=dg[:, 0:1],
                                            in1=dg[:, 0:1], op=ALU.mult)
                    vef = mp_.tile([h, 1], f32, tag="vef", name=f"vef{l}")
                    nc.vector.scalar_tensor_tensor(
                        out=vef, in0=dg[:, 1:2], scalar=float(EPS), in1=musq2,
                        op0=ALU.add, op1=ALU.subtract)
                    sd2 = mp_.tile([h, 1], f32, tag="sd2", name=f"sd2{l}")
                    nc.scalar.activation(sd2, vef, AF.Sqrt)
                    rr = mp_.tile([h, 1], f32, tag="rr", name=f"rr{l}")
                    nc.vector.reciprocal(rr, sd2)
                    s2 = mp_.tile([h, 1], f32, tag="s2", name=f"s2{l}")
                    nc.vector.tensor_tensor(out=s2, in0=rr,
                                            in1=midg_sb[:, l:l + 1], op=ALU.mult)
                    wf = mp_.tile([h, h], f32r, tag="wf", name=f"wf{l}")
                    nc.vector.tensor_scalar_mul(wf, midwt_sb[:, l, :], s2)
                    mt = mp_.tile([h, 1], f32, tag="mt", name=f"mt{l}")
                    nc.vector.tensor_tensor(out=mt, in0=dg[:, 0:1], in1=s2,
                                            op=ALU.mult)
                    t2 = mp_.tile([h, 1], f32, tag="t2", name=f"t2{l}")
                    nc.vector.tensor_tensor(out=t2, in0=midbeta_sb[:, l:l + 1],
                                            in1=mt, op=ALU.subtract)
                    psb2 = mpb.tile([h, 1], f32, tag="scr", name=f"psb2_{l}")
                    nc.tensor.matmul(psb2, midwt_sb[:, l, :], t2,
                                     skip_group_check=True)
                    bias2 = mp_.tile([h, 1], f32, tag="bias2", name=f"bias2{l}")
                    nc.vector.tensor_tensor(out=bias2, in0=psb2,
                                            in1=midbias_sb[:, l:l + 1], op=ALU.add)

                    last = (l == n_layers - 1)
                    h_out_f = h_out.bitcast(f32)
                    for g in range(ngm):
                        sl = slice(g * GWM, (g + 1) * GWM)
                        psm = mps.tile([h, GWM], f32, tag="psm",
                                       name=f"psm{l}_{g}")
                        nc.tensor.matmul(psm[:, 0:512], wf,
                                         h_in[:, g * GWM:g * GWM + 512],
                                         skip_group_check=True)
                        nc.tensor.matmul(psm[:, 512:GWM], wf,
                                         h_in[:, g * GWM + 512:(g + 1) * GWM],
                                         skip_group_check=True)
                        if not last:
                            nc.scalar.activation(h_out[:, sl], psm, AF.Relu,
                                                 bias=bias2,
                                                 accum_out=hs[:, g:g + 1])
                            scr = scrp.tile([h, GWM], f32, tag="sq",
                                            name=f"sq{l}_{g}")
                            nc.vector.scalar_tensor_tensor(
                                out=scr, in0=h_out_f[:, sl], scalar=1.0,
                                in1=h_out_f[:, sl], op0=ALU.mult, op1=ALU.mult,
                                accum_out=hq[:, g:g + 1])
                        else:
                            nc.scalar.activation(h_out[:, sl], psm, AF.Relu,
                                                 bias=bias2)
                            for c in range(GWM // 512):
                                c0 = g * GWM + c * 512
                                pso = mpso.tile([1, 512], f32, tag="pso",
                                                name=f"pso{g}_{c}")
                                nc.tensor.matmul(pso, woutt16,
                                                 h_out[:, c0:c0 + 512],
                                                 skip_group_check=True)
                                nc.scalar.activation(out_row[:, c0:c0 + 512],
                                                     pso, AF.Identity,
                                                     bias=bout_sb[0:1, 0:1])
                    h_in, h_out = h_out, h_in

                nc.sync.dma_start(out_d.rearrange("b o -> o b"), out_row)
            scrp.release()
            sp.release()
            hp.release()

    # patch remote-wait thresholds (Tile's scheduling sim can't model
    # cross-core sem increments; emitted as >=0, fixed to real target here)
    for w_ins, target in wait_patches:
        si = w_ins.sync_info
        si.on_wait[0].wait_value = target
        w_ins.sync_info = si

    nc.compile()
    return nc


def make_in_maps(inputs, n_cores=N_CORES, b_local=B // N_CORES):
    """Host-side layout prep: shard+cast x to fp16 (transposed layout)."""
    x = np.asarray(inputs["x"], np.float32)
    QD = D // 128
    xf = x.astype(F16)
    w_in = np.asarray(inputs["W_in"], np.float32)
    wint = np.ascontiguousarray(w_in.T).reshape(QD, 128, H).transpose(1, 0, 2)
    wint = np.ascontiguousarray(wint)
    bin_ = np.asarray(inputs["b_in"], np.float32).reshape(-1, 1)
    growp = np.ascontiguousarray(
        np.asarray(inputs["bn_gamma_in"], np.float32).reshape(QD, 128).T)
    browp = np.ascontiguousarray(
        np.asarray(inputs["bn_beta_in"], np.float32).reshape(QD, 128).T)
    midwt = np.ascontiguousarray(
        np.asarray(inputs["mid_W"], np.float32).transpose(0, 2, 1))
    midg = np.ascontiguousarray(np.asarray(inputs["mid_gamma"], np.float32).T)
    midbeta = np.ascontiguousarray(np.asarray(inputs["mid_beta"], np.float32).T)
    midbias = np.ascontiguousarray(np.asarray(inputs["mid_b"], np.float32).T)
    woutt = np.ascontiguousarray(np.asarray(inputs["W_out"], np.float32).T)
    bout = np.asarray(inputs["b_out"], np.float32).reshape(1, 1)
    identf = np.eye(128, dtype=np.float32)
    sel16 = np.zeros((2 * N_CORES, 2), np.float32)
    sel16[0::2, 0] = 1.0
    sel16[1::2, 1] = 1.0

    common = dict(wint=wint, bin=bin_, growp=growp, browp=browp, midwt=midwt,
                  midg=midg, midbeta=midbeta, midbias=midbias, woutt=woutt,
                  bout=bout, identf=identf, sel16=sel16)
    in_maps = []
    for cc in range(n_cores):
        m = dict(common)
        shard = xf[cc * b_local:(cc + 1) * b_local]
        m["xt"] = np.ascontiguousarray(shard.T)
        in_maps.append(m)
    return in_maps


@functools.lru_cache(maxsize=1)
def _get_program():
    return build_program()


def kernel(**inputs) -> np.ndarray:
    from concourse.bass_utils import run_bass_kernel_spmd
    nc = _get_program()
    in_maps = make_in_maps(inputs)
    res = run_bass_kernel_spmd(nc, in_maps, core_ids=list(range(N_CORES)))
    out = np.concatenate([res.results[c]["out"] for c in range(N_CORES)], axis=0)
    return out.astype(np.float32)


if __name__ == "__main__":
    nc = build_program(n_cores=2, b_local=2048, d=512, n_layers=2)
    print("built ok:", len(nc.inst_map), "instructions")
